# revision 44
# baseline (speedup 1.0000x reference)
"""Trainium2 Bass kernel for nn_MaskedPosmap2Normal.

Per batch image b and pixel (i,j), the reference computes
    d_k = neighbor_k - center  (k = right, up, left, down; zero-padded)
    normal = sum_k valid_k * (d_k x d_{k+1 mod 4})
    out = normal / max(||normal||, 1e-12)
where valid_k is the AND of the 3 mask bits bracketing directions k, k+1.

Algebraic factorization used here (verified vs the reference):
    G = m_u*du - m_d*dd ,  H = m_r*dr - m_l*dl  (per xyz channel)
    normal = m_c * (H x G)
i.e. ONE cross product instead of four, and the 12 valid-map conv terms
reduce to shifted-mask multiplies.

Sharding: pure data parallel — one batch image per NeuronCore (8 cores).

Layout per core: partition p holds image rows [8p-1 .. 8p+8] (8 output rows
+ 1 halo row each side) so every stencil shift is a free-dim offset.
Columns are processed in chunks of CW with a 2-column halo (per-row pitch
P = CW + 4). The mask (uint8 on device) stays SBUF-resident for the whole
image along with precombined fields mA = m_c*m_u and mB = m_c*m_d.

The default (fused) pipeline runs every elementwise op across all 3 xyz
channels at once (free size 3*8*CW per instruction), computes the
cross-product subtraction n = ca - cb and the |n|^2 accumulation on the
otherwise-idle TensorEngine via (+/-)identity matmuls accumulating in PSUM,
and evaluates 1/||n|| as exp(-0.5*ln(s/256 + 1e-24) - ln 16) on the ACT
engine (Rsqrt/Reciprocal ACT LUTs are banned for accuracy; ln/exp share one
table set). Input loads issue on the SP HWDGE queue, output stores on the
ACT HWDGE queue so stores never head-of-line-block the next chunk's loads.

Numerics: the diff/mask/cross pipeline is kept in fp32 — the cross product
suffers catastrophic cancellation on near-parallel (H, G) pixels, and f16
intermediates there produce O(0.1) absmax errors vs the fp32 reference
(measured on the real inputs). fp32 end-to-end gives absmax ~4e-5.

GPSIMD elementwise offload (2x slower + SBUF port contention with DVE) and
DMA-CCE accumulation (wrong results on real HW) were tried and rejected.
"""

import os

import numpy as np

CH = 3
RPG = 8   # output rows per partition
NG = 10   # rows incl. halo
NCORES = 8

CW = int(os.environ.get("K_CW", "128"))
# comma-separated op-sites to run on GPSIMD: subset of {d,t,x,s,o}
GP_SITES = frozenset(x for x in os.environ.get("K_GP", "").split(",") if x)
FUSE = os.environ.get("K_FUSE", "1") == "1"
# DMA-CCE accumulation for the G/H subtractions: produced WRONG results on
# real hardware (sim-only win) — keep off.
CCE_MODE = os.environ.get("K_CCE", "")  # "", "g", or "gh": DMA-accum subs
CCE = CCE_MODE in ("1", "g", "gh")
CCE_H = CCE_MODE in ("1", "gh")

_CACHE = {}


def _emit(ctx, tc, pm, mk, out, H, W, cw, reps=1):
    import concourse.bass as bass
    from concourse import mybir

    nc = tc.nc
    f32 = mybir.dt.float32
    f16 = mybir.dt.float16
    AF = mybir.ActivationFunctionType
    ALU = mybir.AluOpType

    def eng(site):
        return nc.gpsimd if site in GP_SITES else nc.vector

    NP = H // RPG          # partitions used (128 at full size)
    P = cw + 4             # per-row pitch in a column-chunk tile
    PM = W + 4             # per-row pitch of the resident mask tile
    nchunks = W // cw
    LN16 = float(np.log(16.0))

    def vw(t, pitch, r0, s0, nr=RPG, w=cw):
        return t.rearrange("p (r q) -> p r q", r=NG)[:, r0 : r0 + nr, s0 : s0 + w]

    zrow = {}  # dtype -> zeroed [NP, PM] scratch (for halo-row zeroing via DMA)

    def load_tile(pool, handle, base_off, dt, name, pitch, lo, ncols, soff):
        """Load rows [8p-1 .. 8p+8] x cols [lo .. lo+ncols) into slot soff."""
        t = pool.tile([NP, NG * pitch], dt, name=name, tag=name.split("_")[0])
        tv = t.rearrange("p (r q) -> p r q", r=NG)
        src = bass.AP(handle, base_off + (RPG - 1) * W + lo,
                      [[RPG * W, NP - 2], [W, NG], [1, ncols]])
        nc.sync.dma_start(out=tv[1 : NP - 1, :, soff : soff + ncols], in_=src)
        src0 = bass.AP(handle, base_off + lo, [[W * H, 1], [W, NG - 1], [1, ncols]])
        nc.sync.dma_start(out=tv[0:1, 1:NG, soff : soff + ncols], in_=src0)
        src1 = bass.AP(handle, base_off + (H - (NG - 1)) * W + lo,
                       [[W * H, 1], [W, NG - 1], [1, ncols]])
        nc.sync.dma_start(out=tv[NP - 1 : NP, 0 : NG - 1, soff : soff + ncols],
                          in_=src1)
        z = zrow[dt]
        nc.sync.dma_start(out=tv[0:1, 0:1, :], in_=z[0:1, 0:pitch])
        nc.sync.dma_start(out=tv[NP - 1 : NP, NG - 1 : NG, :], in_=z[0:1, 0:pitch])
        if soff > 0:
            nc.gpsimd.memset(tv[:, :, 0:soff], 0.0)
        if soff + ncols < pitch:
            nc.gpsimd.memset(tv[:, :, soff + ncols : pitch], 0.0)
        return t

    big = cw >= 256
    xin = ctx.enter_context(tc.tile_pool(name="xin", bufs=3 if big else 4))
    mres = ctx.enter_context(tc.tile_pool(name="mres", bufs=1))
    wpool = ctx.enter_context(tc.tile_pool(name="wpool", bufs=4 if big else 5))
    gh = ctx.enter_context(tc.tile_pool(name="gh", bufs=6 if big else 7))
    npool = ctx.enter_context(tc.tile_pool(name="npool", bufs=3 if big else 4))
    spool = ctx.enter_context(tc.tile_pool(name="spool", bufs=3 if big else 5))
    s32pool = ctx.enter_context(tc.tile_pool(name="s32pool", bufs=2))
    opool = ctx.enter_context(tc.tile_pool(name="opool", bufs=3 if big else 4))

    # per-partition bias constants for the ACT ops
    bias_eps = mres.tile([NP, 1], f32, name="bias_eps")
    nc.gpsimd.memset(bias_eps[:], 1e-24)
    bias_ln16 = mres.tile([NP, 1], f32, name="bias_ln16")
    nc.gpsimd.memset(bias_ln16[:], -LN16)

    for dt in (f32, f16, mybir.dt.uint8):
        z = mres.tile([NP, PM], dt, name=f"zrow_{dt.name}")
        nc.gpsimd.memset(z[:], 0.0)
        zrow[dt] = z

    # resident mask (u8): cols [-2 .. W+1] at slots 0..PM-1, and precombined
    # center-folded fields mA = m_c*m_u, mB = m_c*m_d (8 output rows only).
    u8 = mybir.dt.uint8
    mt = load_tile(mres, mk, 0, u8, "mt", PM, 0, W, 2)
    mtv = mt.rearrange("p (r q) -> p r q", r=NG)
    mA = mres.tile([NP, RPG * PM], u8, name="mA")
    mB = mres.tile([NP, RPG * PM], u8, name="mB")
    m8 = lambda t: t.rearrange("p (r q) -> p r q", r=RPG)
    nc.vector.tensor_tensor(m8(mA), mtv[:, 1:9, :], mtv[:, 0:8, :], ALU.mult)
    nc.vector.tensor_tensor(m8(mB), mtv[:, 1:9, :], mtv[:, 2:10, :], ALU.mult)

    for rep in range(reps):
      for k0 in range(nchunks):
        k = rep * nchunks + k0
        j0 = k0 * cw
        lo = max(j0 - 2, 0)
        hi = min(j0 + cw + 1, W - 1)
        ncols = hi - lo + 1
        soff = lo - (j0 - 2)

        xts = [load_tile(xin, pm, c * H * W, f32, f"x_{k}_{c}", P, lo, ncols, soff)
               for c in range(CH)]

        # mask views for this chunk (slot = col + 2 in the resident tiles)
        mAv = m8(mA)[:, :, j0 + 2 : j0 + 2 + cw]
        mBv = m8(mB)[:, :, j0 + 2 : j0 + 2 + cw]
        mR = mtv[:, 1:9, j0 + 3 : j0 + 3 + cw]
        mL = mtv[:, 1:9, j0 + 1 : j0 + 1 + cw]

        Gs, Hs = [], []
        for c in range(CH):
            xt = xts[c]
            xC = vw(xt, P, 1, 2)
            xU = vw(xt, P, 0, 2)
            xD = vw(xt, P, 2, 2)
            xR = vw(xt, P, 1, 3)
            xL = vw(xt, P, 1, 1)

            w3 = lambda t: t.rearrange("p (r q) -> p r q", r=RPG)

            def wt(nm):
                return wpool.tile([NP, RPG * cw], f32, name=f"{nm}_{k}_{c}", tag="w")

            du = wt("du"); eng("d").tensor_sub(w3(du), xU, xC)
            dd = wt("dd"); eng("d").tensor_sub(w3(dd), xD, xC)
            t1 = wt("t1"); eng("t").tensor_tensor(w3(t1), mAv, w3(du), ALU.mult)
            t2 = wt("t2"); eng("t").tensor_tensor(w3(t2), mBv, w3(dd), ALU.mult)
            G = gh.tile([NP, RPG * cw], f32, name=f"G_{k}_{c}", tag="gh")
            eng("g").tensor_sub(G[:], t1[:], t2[:])

            dr = wt("dr"); eng("d").tensor_sub(w3(dr), xR, xC)
            dl = wt("dl"); eng("d").tensor_sub(w3(dl), xL, xC)
            t3 = wt("t3"); eng("t").tensor_tensor(w3(t3), mR, w3(dr), ALU.mult)
            t4 = wt("t4"); eng("t").tensor_tensor(w3(t4), mL, w3(dl), ALU.mult)
            Ht = gh.tile([NP, RPG * cw], f32, name=f"H_{k}_{c}", tag="gh")
            eng("g").tensor_sub(Ht[:], t3[:], t4[:])
            Gs.append(G)
            Hs.append(Ht)

        # n = H x G
        ns = []
        for c in range(CH):
            a, b = (c + 1) % 3, (c + 2) % 3
            ta = wpool.tile([NP, RPG * cw], f32, name=f"ca_{k}_{c}", tag="w")
            eng("x").tensor_tensor(ta[:], Hs[a][:], Gs[b][:], ALU.mult)
            tb = wpool.tile([NP, RPG * cw], f32, name=f"cb_{k}_{c}", tag="w")
            eng("x").tensor_tensor(tb[:], Hs[b][:], Gs[a][:], ALU.mult)
            n_c = npool.tile([NP, RPG * cw], f32, name=f"n_{k}_{c}", tag="n")
            eng("n").tensor_sub(n_c[:], ta[:], tb[:])
            ns.append(n_c)

        # r = 1/sqrt(s/256 + 1e-24)/16 = 1/sqrt(s + 2.56e-22)
        def sq_tile(c):
            s_c = spool.tile([NP, RPG * cw], f32, name=f"sq_{k}_{c}", tag="s")
            nc.scalar.activation(s_c[:], ns[c][:], AF.Square, scale=0.0625)
            return s_c
        sq0, sq1 = sq_tile(0), sq_tile(1)
        s01 = spool.tile([NP, RPG * cw], f32, name=f"s01_{k}", tag="s")
        eng("s").tensor_add(s01[:], sq0[:], sq1[:])
        sq2 = sq_tile(2)
        s2 = spool.tile([NP, RPG * cw], f32, name=f"s2_{k}", tag="s")
        eng("s").tensor_add(s2[:], s01[:], sq2[:])
        lns = s32pool.tile([NP, RPG * cw], f32, name=f"lns_{k}", tag="s32")
        nc.scalar.activation(lns[:], s2[:], AF.Ln, bias=bias_eps[:])
        r = s32pool.tile([NP, RPG * cw], f32, name=f"r_{k}", tag="s32")
        nc.scalar.activation(r[:], lns[:], AF.Exp, scale=-0.5, bias=bias_ln16[:])
        for c in range(CH):
            o = opool.tile([NP, RPG * cw], f32, name=f"o_{k}_{c}", tag="o")
            eng("o").tensor_tensor(o[:], ns[c][:], r[:], ALU.mult)
            dst = bass.AP(out, c * H * W + j0, [[RPG * W, NP], [W, RPG], [1, cw]])
            nc.sync.dma_start(out=dst, in_=o.rearrange("p (r q) -> p r q", r=RPG))


def _emit_fused(ctx, tc, pm, mk, out, H, W, cw, reps=1):
    """Channel-fused variant: one op spans all 3 xyz channels (N = 3*8*cw),
    and the cross-product subtraction + |n|^2 accumulation run on the idle
    TensorEngine via identity matmuls accumulating in PSUM."""
    import concourse.bass as bass
    from concourse import mybir
    from concourse.masks import make_identity

    nc = tc.nc
    f32 = mybir.dt.float32
    u8 = mybir.dt.uint8
    AF = mybir.ActivationFunctionType
    ALU = mybir.AluOpType

    NP = H // RPG
    P = cw + 4
    PM = W + 4
    NF = CH * RPG * cw          # fused free size (3*8*cw)
    SEG = RPG * cw              # per-channel block inside a fused tile
    nchunks = W // cw
    LN16 = float(np.log(16.0))

    def bufs(name, dflt):
        return int(os.environ.get(f"K_B_{name}", str(dflt)))

    xin = ctx.enter_context(tc.tile_pool(name="xin", bufs=bufs("x", 2)))
    mres = ctx.enter_context(tc.tile_pool(name="mres", bufs=1))
    wpool = ctx.enter_context(tc.tile_pool(name="wpool", bufs=bufs("w", 4)))
    gh = ctx.enter_context(tc.tile_pool(name="gh", bufs=bufs("gh", 2)))
    sqpool = ctx.enter_context(tc.tile_pool(name="sqpool", bufs=bufs("sq", 1)))
    s32pool = ctx.enter_context(tc.tile_pool(name="s32pool", bufs=2))
    opool = ctx.enter_context(tc.tile_pool(name="opool", bufs=bufs("o", 2)))
    psum = ctx.enter_context(tc.tile_pool(name="psum", bufs=1, space="PSUM"))

    bias_eps = mres.tile([NP, 1], f32, name="bias_eps")
    nc.gpsimd.memset(bias_eps[:], 1e-24)
    bias_ln16 = mres.tile([NP, 1], f32, name="bias_ln16")
    nc.gpsimd.memset(bias_ln16[:], -LN16)
    zrow = mres.tile([NP, 3 * P], f32, name="zrow32")
    nc.gpsimd.memset(zrow[:], 0.0)
    zrow8 = mres.tile([NP, PM], u8, name="zrow8")
    nc.gpsimd.memset(zrow8[:], 0.0)

    ident = mres.tile([NP, NP], f32, name="ident")
    make_identity(nc, ident[:])
    nident = mres.tile([NP, NP], f32, name="nident")
    nc.vector.tensor_scalar_mul(nident[:], ident[:], -1.0)

    # resident mask (u8) + precombined center-folded fields
    mt = mres.tile([NP, NG * PM], u8, name="mt")
    mtv = mt.rearrange("p (r q) -> p r q", r=NG)
    src = bass.AP(mk, (RPG - 1) * W, [[RPG * W, NP - 2], [W, NG], [1, W]])
    nc.sync.dma_start(out=mtv[1 : NP - 1, :, 2 : 2 + W], in_=src)
    src0 = bass.AP(mk, 0, [[W * H, 1], [W, NG - 1], [1, W]])
    nc.sync.dma_start(out=mtv[0:1, 1:NG, 2 : 2 + W], in_=src0)
    src1 = bass.AP(mk, (H - (NG - 1)) * W, [[W * H, 1], [W, NG - 1], [1, W]])
    nc.sync.dma_start(out=mtv[NP - 1 : NP, 0 : NG - 1, 2 : 2 + W], in_=src1)
    nc.sync.dma_start(out=mtv[0:1, 0:1, :], in_=zrow8[0:1, 0:PM])
    nc.sync.dma_start(out=mtv[NP - 1 : NP, NG - 1 : NG, :], in_=zrow8[0:1, 0:PM])
    nc.gpsimd.memset(mtv[:, :, 0:2], 0)
    nc.gpsimd.memset(mtv[:, :, PM - 2 : PM], 0)

    i8 = mybir.dt.int8
    mB_dt = i8 if CCE else u8
    mA = mres.tile([NP, RPG * PM], u8, name="mA")
    mB = mres.tile([NP, RPG * PM], mB_dt, name="mB")
    m8 = lambda t: t.rearrange("p (r q) -> p r q", r=RPG)
    nc.vector.tensor_tensor(m8(mA), mtv[:, 1:9, :], mtv[:, 0:8, :], ALU.mult)
    nc.vector.tensor_tensor(m8(mB), mtv[:, 1:9, :], mtv[:, 2:10, :], ALU.mult)
    if CCE:
        # negated mask fields so G/H become pure additions (DMA CCE accum)
        nc.vector.tensor_scalar_mul(mB[:], mB[:], -1.0)
        mLn = mres.tile([NP, RPG * PM], i8, name="mLn")
        nc.vector.tensor_scalar_mul(m8(mLn), mtv[:, 1:9, :], -1.0)

    def bc3(view):  # [NP, 8, cw] -> broadcast [NP, 3, 8, cw]
        v = view.unsqueeze(1)
        return v.to_broadcast([NP, CH, RPG, cw])

    for rep in range(reps):
      for k0 in range(nchunks):
        k = rep * nchunks + k0
        j0 = k0 * cw
        lo = max(j0 - 2, 0)
        hi = min(j0 + cw + 1, W - 1)
        ncols = hi - lo + 1
        soff = lo - (j0 - 2)

        # fused X tile [NP, 3, NG, P]; per-channel DMAs (balancer caps at 3 dims)
        xt = xin.tile([NP, CH * NG * P], f32, name=f"x_{k}", tag="x")
        xt4 = xt.rearrange("p (c r q) -> p c r q", c=CH, r=NG)
        for c in range(CH):
            base = c * H * W
            tv = xt4[:, c]
            src = bass.AP(pm, base + (RPG - 1) * W + lo,
                          [[RPG * W, NP - 2], [W, NG], [1, ncols]])
            nc.sync.dma_start(out=tv[1 : NP - 1, :, soff : soff + ncols], in_=src)
            src0 = bass.AP(pm, base + lo, [[W * H, 1], [W, NG - 1], [1, ncols]])
            nc.sync.dma_start(out=tv[0:1, 1:NG, soff : soff + ncols], in_=src0)
            src1 = bass.AP(pm, base + (H - (NG - 1)) * W + lo,
                           [[W * H, 1], [W, NG - 1], [1, ncols]])
            nc.sync.dma_start(out=tv[NP - 1 : NP, 0 : NG - 1, soff : soff + ncols],
                              in_=src1)
            nc.sync.dma_start(out=tv[0:1, 0:1, :], in_=zrow[0:1, 0:P])
            nc.sync.dma_start(out=tv[NP - 1 : NP, NG - 1 : NG, :],
                              in_=zrow[0:1, 0:P])
        if soff > 0:
            nc.gpsimd.memset(xt4[:, :, :, 0:soff], 0.0)
        if soff + ncols < P:
            nc.gpsimd.memset(xt4[:, :, :, soff + ncols : P], 0.0)

        xC = xt4[:, :, 1:9, 2 : 2 + cw]
        xU = xt4[:, :, 0:8, 2 : 2 + cw]
        xD = xt4[:, :, 2:10, 2 : 2 + cw]
        xR = xt4[:, :, 1:9, 3 : 3 + cw]
        xL = xt4[:, :, 1:9, 1 : 1 + cw]

        mAv = bc3(m8(mA)[:, :, j0 + 2 : j0 + 2 + cw])
        mBv = bc3(m8(mB)[:, :, j0 + 2 : j0 + 2 + cw])
        mR = bc3(mtv[:, 1:9, j0 + 3 : j0 + 3 + cw])
        if CCE:
            mL = bc3(m8(mLn)[:, :, j0 + 1 : j0 + 1 + cw])
        else:
            mL = bc3(mtv[:, 1:9, j0 + 1 : j0 + 1 + cw])

        def wt(nm):
            return wpool.tile([NP, NF], f32, name=f"{nm}_{k}", tag="w")

        w4 = lambda t: t.rearrange("p (c r q) -> p c r q", c=CH, r=RPG)

        du = wt("du"); nc.vector.tensor_sub(w4(du), xU, xC)
        dd = wt("dd"); nc.vector.tensor_sub(w4(dd), xD, xC)
        G = gh.tile([NP, NF], f32, name=f"G_{k}", tag="gh")
        Ht = gh.tile([NP, NF], f32, name=f"H_{k}", tag="gh")
        if CCE:
            # t1 written straight into G; t2 (sign-negated via mB=-mask) is
            # folded in by a DMA-engine CCE accumulation: G += t2.
            nc.vector.tensor_tensor(w4(G), mAv, w4(du), ALU.mult)
            t2 = wt("t2"); nc.vector.tensor_tensor(w4(t2), mBv, w4(dd), ALU.mult)
            nc.gpsimd.dma_start(out=G[:], in_=t2[:], accum_op=ALU.add)
        else:
            t1 = wt("t1"); nc.vector.tensor_tensor(w4(t1), mAv, w4(du), ALU.mult)
            t2 = wt("t2"); nc.vector.tensor_tensor(w4(t2), mBv, w4(dd), ALU.mult)
            nc.vector.tensor_sub(G[:], t1[:], t2[:])

        dr = wt("dr"); nc.vector.tensor_sub(w4(dr), xR, xC)
        dl = wt("dl"); nc.vector.tensor_sub(w4(dl), xL, xC)
        if CCE_H:
            nc.vector.tensor_tensor(w4(Ht), mR, w4(dr), ALU.mult)
            t4 = wt("t4"); nc.vector.tensor_tensor(w4(t4), mL, w4(dl), ALU.mult)
            nc.gpsimd.dma_start(out=Ht[:], in_=t4[:], accum_op=ALU.add)
        else:
            t3 = wt("t3"); nc.vector.tensor_tensor(w4(t3), mR, w4(dr), ALU.mult)
            t4n = wt("t4")
            if CCE:  # mLn is negated: t4n = -mL*dl, so H = t3 + t4n
                nc.vector.tensor_tensor(w4(t4n), mL, w4(dl), ALU.mult)
                nc.vector.tensor_add(Ht[:], t3[:], t4n[:])
            else:
                nc.vector.tensor_tensor(w4(t4n), mL, w4(dl), ALU.mult)
                nc.vector.tensor_sub(Ht[:], t3[:], t4n[:])

        # cross-product muls into fused ca/cb, then n = ca - cb on TensorE
        ca = wt("ca")
        cb = wt("cb")
        for c in range(CH):
            a, b = (c + 1) % 3, (c + 2) % 3
            sl = lambda t, i: t[:, i * SEG : (i + 1) * SEG]
            nc.vector.tensor_tensor(sl(ca, c), sl(Ht, a), sl(G, b), ALU.mult)
            nc.vector.tensor_tensor(sl(cb, c), sl(Ht, b), sl(G, a), ALU.mult)

        n_ps = psum.tile([NP, NF], f32, name=f"n_{k}", tag="n")
        for s0 in range(0, NF, 512):
            sw = min(512, NF - s0)
            nc.tensor.matmul(n_ps[:, s0 : s0 + sw], ident[:],
                             ca[:, s0 : s0 + sw], start=True, stop=False)
            nc.tensor.matmul(n_ps[:, s0 : s0 + sw], nident[:],
                             cb[:, s0 : s0 + sw], start=False, stop=True)

        # |n|^2 via ACT squares (scaled by 1/256) + TensorE accumulation
        sq = sqpool.tile([NP, NF], f32, name=f"sq_{k}", tag="sq")
        nc.scalar.activation(sq[:], n_ps[:], AF.Square, scale=0.0625)
        s_ps = psum.tile([NP, SEG], f32, name=f"s_{k}", tag="s")
        for s0 in range(0, SEG, 512):
            sw = min(512, SEG - s0)
            for c in range(CH):
                nc.tensor.matmul(s_ps[:, s0 : s0 + sw], ident[:],
                                 sq[:, c * SEG + s0 : c * SEG + s0 + sw],
                                 start=(c == 0), stop=(c == CH - 1))

        lns = s32pool.tile([NP, SEG], f32, name=f"lns_{k}", tag="s32")
        nc.scalar.activation(lns[:], s_ps[:], AF.Ln, bias=bias_eps[:])
        r = s32pool.tile([NP, SEG], f32, name=f"r_{k}", tag="s32")
        nc.scalar.activation(r[:], lns[:], AF.Exp, scale=-0.5, bias=bias_ln16[:])

        o = opool.tile([NP, NF], f32, name=f"o_{k}", tag="o")
        rb = r.unsqueeze(1).to_broadcast([NP, CH, SEG])
        nc.vector.tensor_tensor(o.rearrange("p (c q) -> p c q", c=CH),
                                n_ps.rearrange("p (c q) -> p c q", c=CH),
                                rb, ALU.mult)
        o4 = o.rearrange("p (c r q) -> p c r q", c=CH, r=RPG)
        for c in range(CH):
            dst = bass.AP(out, c * H * W + j0,
                          [[RPG * W, NP], [W, RPG], [1, cw]])
            nc.scalar.dma_start(out=dst, in_=o4[:, c])


def build(H=1024, W=1024, cw=None, reps=1):
    cw = cw or CW
    key = (H, W, cw, reps)
    if key in _CACHE:
        return _CACHE[key]
    from contextlib import ExitStack

    import concourse.tile as tile
    from concourse import bacc, mybir

    nc = bacc.Bacc("TRN2", target_bir_lowering=False, debug=False,
                   num_devices=NCORES)
    pm = nc.dram_tensor("posmap", [CH, H, W], mybir.dt.float32,
                        kind="ExternalInput")
    mk = nc.dram_tensor("mask", [H, W], mybir.dt.uint8, kind="ExternalInput")
    out = nc.dram_tensor("out", [CH, H, W], mybir.dt.float32,
                         kind="ExternalOutput")
    with tile.TileContext(nc) as tc:
        with ExitStack() as ctx:
            if FUSE:
                _emit_fused(ctx, tc, pm, mk, out, H, W, cw, reps)
            else:
                _emit(ctx, tc, pm, mk, out, H, W, cw, reps)
    nc.compile()
    _CACHE[key] = nc
    return nc


def kernel(posmap: np.ndarray, mask: np.ndarray, _trace: bool = False):
    nc = build(posmap.shape[2], posmap.shape[3])
    from concourse.bass_utils import run_bass_kernel_spmd

    mask_u8 = np.ascontiguousarray(mask.astype(np.uint8))
    nb = posmap.shape[0]
    in_maps = [
        {"posmap": np.ascontiguousarray(posmap[b]), "mask": mask_u8}
        for b in range(nb)
    ]
    try:
        res = run_bass_kernel_spmd(nc, in_maps, core_ids=list(range(nb)),
                                   trace=_trace)
    except ModuleNotFoundError:
        res = run_bass_kernel_spmd(nc, in_maps, core_ids=list(range(nb)),
                                   trace=False)
    out = np.stack([res.results[b]["out"] for b in range(nb)], axis=0)
    if _trace:
        kernel.last_exec_time_ns = res.exec_time_ns
        kernel.last_trace = res.instructions_and_trace
    return out
